# revision 42
# baseline (speedup 1.0000x reference)
"""Trainium2 Bass kernel for ConvNetWithGlobalPooling (batch-parallel grouped CNN).

Per-sample network: 3x(3x3 SAME conv + per-sample bias + relu) ->
global mean pool -> per-sample outer product with fc vector + bias.

Sharding: pure data parallel, 4 samples per core across 8 cores.

Device strategy (per sample), v2 = fp8 DoubleRow edition:
  - Activations live in SBUF in "padded flat" layout with ROW STRIDE 80
    (not 66): a [C, 66*80] fp8 image, 64 valid cols + borders + 14 garbage
    cols per row. Stride 80 makes the dy-shift delta (+-80) and the
    dy=0 <-> dy=2 delta (160) multiples of 16, which is the alignment the
    DoubleRow pair-axis step requires. Matmul rhs uses a [rows x 80][64]
    2-level AP so only valid columns stream.
  - conv2/conv3 run in fp8 e4m3 with perf_mode=DoubleRow: the PE packs two
    fp8 weights per cell (K=256 per pass, 2 MACs/cycle), so each DR matmul
    covers TWO 3x3 shifts. Shift pairing is (dy=0,dx) with (dy=2,dx) via the
    rhs pair axis (step 160 = 2 image rows).
  - conv2 (Cin=64): pad1 holds conv1's output twice - rows 0:64 normal,
    rows 64:128 shifted down one row - so one DR matmul covers FOUR shifts
    ((0,j),(1,j) via the row halves) x ((+0,+160) via the pair axis ->
    (2,j) with zero bottom weights). 3 DR matmuls total (vs 9 plain).
  - conv3 (Cin=128): 3 DR matmuls (pairs (0,j)+(2,j)) + 3 plain fp8
    matmuls ((1,j)) per Cout-half. 6 streams instead of 9.
  - Quantization scales a1=2, a2=8, a3=32 folded into weights/biases on
    host (w1*=a1, w2*=a2/a1, w3*=a3/a2, fc/=a3*4096); fp8 tensors never
    need a device-side dequant op.
  - conv1 stays bf16 (K=27 host-built im2col), drains write fp8 pad1.
  - Engine split: PE matmuls; ACT does conv1-top drain and conv3 drains
    (accum_out = free global pooling); DVE does conv1-bottom + conv2
    drains and the fc stage.
  - pad buffers are persistent (2 x pad1, 2 x pad2 ping-pong across
    samples); their zero borders are memset ONCE at kernel start
    (border-only strided memsets), since drains never touch border bytes.
  - Emission is software-pipelined: conv1 of sample s+1 sits between
    conv3's two Cout halves of sample s so the PE never starves.
"""

import os
import sys

sys.path.insert(0, "/opt/trn_rl_repo")

import numpy as np

import concourse.bass as bass
import concourse.bacc as bacc
import concourse.tile as tile
from concourse import mybir
from concourse.ap import AP
from concourse.bass_utils import run_bass_kernel_spmd

F32 = mybir.dt.float32
BF16 = mybir.dt.bfloat16
F8 = mybir.dt.float8e4
DRMODE = mybir.MatmulPerfMode.DoubleRow
RELU = mybir.ActivationFunctionType.Relu
ADD = mybir.AluOpType.add
MAX = mybir.AluOpType.max

B = 32
N_CORES = 8
SPC = B // N_CORES  # samples per core
H = W = 64
# conv1 im2col keeps the old 66-stride layout (host-built)
PW1 = 66
NPIX1 = 66 * 66  # 4356
# fp8 activation pads use row stride 80 (16-aligned shift deltas)
PW = 80
PH = 66
PADLEN = PH * PW  # 5280
ROWS_PER_CHUNK = 8
RCHUNKS = [(1 + 8 * k, 8) for k in range(8)]
# stride-80 deltas: dy=0: -81,-80,-79 ; dy=1: -1,0,1 ; dy=2: 79,80,81
D0 = [-81, -80, -79]  # delta of (dy=0, dx=j)
D1 = [-1, 0, 1]  # delta of (dy=1, dx=j)
# pad2 second-copy offset: = 15 (mod 16) so the DoubleRow pair step
# OB2+1 (pairing (1,0) with (1,1)) is 16-aligned
OB2 = 5295
# scales (powers of 2, folded into weights/biases host-side)
A1, A2, A3 = 2.0, 8.0, 32.0
N_WARMUP = 12  # dummy matmuls at t~5us to lift the HAM clock gate early


def _rv(ap, off, nrows, pw):
    """[P, nrows, 64] view of a padded-flat buffer starting at `off`."""
    return ap[:, off : off + nrows * pw].rearrange("p (r c) -> p r c", c=pw)[
        :, :, 0:64
    ]


def _dr_rhs(base_ap, off, nrows, pstep=160):
    """DoubleRow rhs: [P][2 (step pstep)][nrows (step 80)][64 (step 1)]."""
    return AP(
        tensor=base_ap.tensor,
        offset=base_ap.offset + off,
        ap=[list(base_ap.ap[0]), [pstep, 2], [PW, nrows], [1, 64]],
    )


def build_nc():
    nc = bacc.Bacc(
        "TRN2",
        target_bir_lowering=False,
        debug=False,
        num_devices=N_CORES,
    )
    xim_d = nc.declare_dram_parameter("xim", [SPC, 27, NPIX1], F8, isOutput=False)
    w1_d = nc.declare_dram_parameter("w1", [SPC, 27, 64], F8, isOutput=False)
    w2_d = nc.declare_dram_parameter("w2", [SPC, 128, 3 * 2 * 128], F8, isOutput=False)
    w3_d = nc.declare_dram_parameter("w3", [SPC, 128, 2 * 1152], F8, isOutput=False)
    sm_d = nc.declare_dram_parameter("sm", [SPC, 128, 24], F32, isOutput=False)
    out_d = nc.declare_dram_parameter("out", [SPC, 128, 20], F32, isOutput=True)

    with tile.TileContext(nc) as tc:
        with (
            tc.tile_pool(name="sb", bufs=2) as sb,
            tc.tile_pool(name="psum", bufs=2, space="PSUM") as psum,
        ):
            wpool = apool = spool = scrpool = sb

            class _PP:
                @staticmethod
                def tile(shape, dt_, name, tag):
                    return sb.tile(shape, dt_, name=name, tag=tag, bufs=1)

            ppool = _PP()
            # persistent ping-pong pad buffers; borders zeroed once.
            # pad2 holds TWO copies (second at +OB2) so conv3 can DR-pair
            # (1,0) with (1,1) across them.
            pads1 = []
            pads2 = []
            for i in range(2):
                pads1.append(
                    ppool.tile([128, PADLEN], F8, name=f"pad1_{i}", tag=f"pad1_{i}")
                )
                pads2.append(
                    ppool.tile(
                        [128, OB2 + PADLEN], F8, name=f"pad2_{i}", tag=f"pad2_{i}"
                    )
                )

            def border_memset(t, eng, off0=0):
                a = t[:]
                # front: padded row 0 + slack  [0, 82)
                eng.memset(a[:, off0 : off0 + 82], 0.0)
                # per-row right border + garbage cols + next left border:
                # rows 1..64, 16 elems starting at r*80+66
                run = AP(
                    tensor=a.tensor,
                    offset=a.offset + off0 + 146,
                    ap=[list(a.ap[0]), [PW, 64], [1, 16]],
                )
                eng.memset(run, 0.0)
                # tail: everything past the last interior write [5106, 5280)
                eng.memset(a[:, off0 + 5106 : off0 + PADLEN], 0.0)

            # HAM warmup: the PE clock gate needs ~3.4us of sustained matmul
            # activity to open to full rate. Real matmuls can't start until
            # the first DMAs land (~10us); burn the idle window on dummy
            # matmuls over a zeroed buffer so the real stream starts warm.
            # The warm memset is FIRST in the DVE queue so warmups start asap.
            # warmups read an UNINITIALIZED dummy tile: no data dependency,
            # so the PE starts the moment its engine program is loaded.
            # (garbage fp8 -> garbage PSUM, cleared later by start=True MMs)
            dummy = ppool.tile([128, 512], F8, name="dummy", tag="dummy")
            nc.vector.memset(dummy[:], 0.0)
            for _ in range(N_WARMUP):
                psw = psum.tile([128, 512], F32, name="ps3", tag="ps3", bufs=3)
                nc.tensor.matmul(
                    psw[:], dummy[:, 0:128], dummy[:], start=True, stop=True
                )
            warm = ppool.tile([128, 512], F8, name="warm", tag="warm")
            nc.vector.memset(warm[:], 0.0)
            zeros = warm  # [128, 512] of zeros, in1 operand for stt drains

            # sample 0's pad1 on DVE (fast, unblocks conv1 drain); pad2's
            # memsets are emitted after the first conv1 pairs so they don't
            # delay the pair drains (whose PSUM-bank WAR gates the PE FIFO);
            # sample 1's pads queue behind the first xim DMA spans on gpsimd
            border_memset(pads1[0], nc.vector)

            T = [None] * SPC  # per-sample tile dict

            def emit_loads_xim(s):
                # xim/w1 live twice (partitions 0:27 and 32:59) so TWO conv1
                # chunks can run concurrently in PE row-groups q0/q1.
                # NOTE: the "scalar" DMA queue is issued by the ACT engine,
                # which is busy with drains -- only sync (SP) and gpsimd
                # queues are used so DMA issue never delays a drain.
                # Each conv1 slice only reads 4 disjoint 527-elem windows
                # (every other chunk), so load each slice with ONE strided
                # window DMA: half the bytes, one queue slot.
                t = {}
                t["pad1"] = pads1[s % 2]
                t["pad2"] = pads2[s % 2]
                t["xim"] = apool.tile([59, NPIX1], F8, name="xim", tag="xim")
                t["w1"] = wpool.tile([59, 64], F8, name="w1", tag="w1")

                def _winview(base_ap, start):
                    return AP(
                        tensor=base_ap.tensor,
                        offset=base_ap.offset + start,
                        ap=[list(base_ap.ap[0]), [1056, 4], [1, 527]],
                    )

                nc.sync.dma_start(
                    _winview(t["xim"][0:27, :], 67), _winview(xim_d[s], 67)
                )
                nc.gpsimd.dma_start(
                    _winview(t["xim"][32:59, :], 595), _winview(xim_d[s], 595)
                )
                nc.sync.dma_start(t["w1"][0:27, :], w1_d[s])
                nc.gpsimd.dma_start(t["w1"][32:59, :], w1_d[s])
                return t

            def emit_loads_rest(s, t):
                sm = spool.tile([128, 24], F32, name="sm", tag="sm")
                t["b1"] = sm[0:64, 0:1]
                t["b2"] = sm[:, 1:2]
                t["b3"] = sm[:, 2:4]
                t["fcb"] = sm[:, 4:24]
                t["w2"] = wpool.tile([128, 3 * 2 * 128], F8, name="w2", tag="w2")
                t["w3"] = wpool.tile([128, 2 * 1152], F8, name="w3", tag="w3")
                nc.sync.dma_start(t["w2"][:], w2_d[s])
                nc.sync.dma_start(sm[:], sm_d[s])
                nc.sync.dma_start(t["w3"][:], w3_d[s])
                return t

            def emit_conv1_pair(t, k):
                # chunks k and k+1 run CONCURRENTLY: row-groups q0/q32 and
                # col-groups 0/64, sharing ONE [128,512] psum bank (chunk A
                # -> partitions 0:64, chunk B -> 64:128)
                n = 512
                ps1 = psum.tile([128, n], F32, name="ps1", tag="ps1")
                for i, (r0, nrows) in enumerate((RCHUNKS[k], RCHUNKS[k + 1])):
                    lhsT = t["w1"][32 * i : 32 * i + 27, :]
                    rhs = _rv(t["xim"][32 * i : 32 * i + 27, :],
                              r0 * PW1 + 1, nrows, PW1)
                    nc.tensor.matmul(
                        ps1[64 * i : 64 * i + 64, :], lhsT, rhs,
                        start=True, stop=True,
                        tile_position=(32 * i, 64 * i),
                    )
                for i, (r0, nrows) in enumerate((RCHUNKS[k], RCHUNKS[k + 1])):
                    src = ps1[64 * i : 64 * i + 64, :].rearrange(
                        "p (r c) -> p r c", c=64
                    )
                    base = r0 * PW
                    dst_t = _rv(t["pad1"][0:64, :], base + 2, nrows, PW)
                    nc.scalar.activation(dst_t, src, RELU, bias=t["b1"][:, 0:1])
                    dst_b = _rv(t["pad1"][64:128, :], base + 2 - PW, nrows, PW)
                    nc.vector.tensor_scalar(
                        dst_b, src, t["b1"][:, 0:1], 0.0, op0=ADD, op1=MAX
                    )

            def emit_conv1(t):
                for k in range(0, 8, 2):
                    emit_conv1_pair(t, k)

            def emit_conv2_chunk(t, r0, nrows, k):
                # 3 fp8 DoubleRow matmuls, each covering 4 shifts:
                # (0,j),(1,j) via pad1's row halves x (+0,+160) pair axis
                # -> (2,j),(3,j=garbage, zero weights).
                # Drains: copy A on ACT; copy B alternates DVE/ACT so
                # neither engine saturates in the interleaved phase.
                n = nrows * 64
                base = r0 * PW
                ps2 = psum.tile([128, n], F32, name="ps2", tag="ps2", bufs=3)
                p1 = t["pad1"][:]
                for j in range(3):
                    lhsT = t["w2"][:, j * 256 : (j + 1) * 256].rearrange(
                        "p (o m) -> p o m", o=2
                    )
                    rhs = _dr_rhs(p1, base + 2 + D0[j], nrows)
                    nc.tensor.matmul(
                        ps2[:], lhsT, rhs,
                        start=(j == 0), stop=(j == 2), perf_mode=DRMODE,
                    )
                src = ps2[:].rearrange("p (r c) -> p r c", c=64)
                dst = _rv(t["pad2"], base + 2, nrows, PW)
                nc.scalar.activation(dst, src, RELU, bias=t["b2"][:, 0:1])
                dst2 = _rv(t["pad2"], OB2 + base + 2, nrows, PW)
                if k % 2 == 0:
                    nc.vector.tensor_scalar(
                        dst2, src, t["b2"][:, 0:1], 0.0, op0=ADD, op1=MAX
                    )
                else:
                    nc.scalar.activation(dst2, src, RELU, bias=t["b2"][:, 0:1])

            def emit_conv3_chunk(t, h, k, drain_eng):
                # 4 DR pairs ((0,j)+(2,j) via step 160; (1,0)+(1,1) via the
                # second pad2 copy at step OB2+1) + 1 plain single ((1,2));
                # relu+bias+pooled-sum drain on ACT (activation+accum_out)
                # or DVE (tensor_scalar+accum_out), chosen per phase.
                p2 = t["pad2"][:]
                r0, nrows = RCHUNKS[k]
                n = nrows * 64
                base = r0 * PW
                ps3 = psum.tile([128, n], F32, name="ps3", tag="ps3", bufs=3)
                lhsT_s = t["w3"][:, h * 1152 + 1024 : h * 1152 + 1152]
                rhs_s = _rv(p2, base + 2 + D1[2], nrows, PW)
                nc.tensor.matmul(ps3[:], lhsT_s, rhs_s, start=True, stop=False)
                for j in range(3):
                    lhsT = t["w3"][
                        :, h * 1152 + j * 256 : h * 1152 + (j + 1) * 256
                    ].rearrange("p (o m) -> p o m", o=2)
                    rhs = _dr_rhs(p2, base + 2 + D0[j], nrows)
                    nc.tensor.matmul(
                        ps3[:], lhsT, rhs,
                        start=False, stop=False, perf_mode=DRMODE,
                    )
                lhsT4 = t["w3"][
                    :, h * 1152 + 768 : h * 1152 + 1024
                ].rearrange("p (o m) -> p o m", o=2)
                rhs4 = _dr_rhs(p2, base + 2 + D1[0], nrows, pstep=OB2 + 1)
                nc.tensor.matmul(
                    ps3[:], lhsT4, rhs4, start=False, stop=True,
                    perf_mode=DRMODE,
                )
                idx = h * 8 + k
                scr = scrpool.tile([128, n], F32, name="scr", tag="scr")
                if drain_eng == "dve":
                    # (psum + bias) MAX zeros, accum_out = sum(out):
                    # one-instruction relu+bias+pooling on DVE
                    nc.vector.scalar_tensor_tensor(
                        scr[:], ps3[:], t["b3"][:, h : h + 1], zeros[:],
                        op0=ADD, op1=MAX,
                        accum_out=t["acc"][:, idx : idx + 1],
                    )
                else:
                    nc.scalar.activation(
                        scr[:], ps3[:], RELU,
                        bias=t["b3"][:, h : h + 1],
                        accum_out=t["acc"][:, idx : idx + 1],
                    )

            def emit_fc_half(s, t, h):
                # per-half fc epilogue: lets the last sample's h0 output ship
                # while h1 matmuls still run, shortening the serial tail
                pooled = spool.tile([128, 1], F32, name="pooledh", tag="pooledh")
                nc.vector.tensor_reduce(
                    pooled[:],
                    t["acc"][:, h * 8 : (h + 1) * 8],
                    axis=mybir.AxisListType.X,
                    op=ADD,
                )
                outh = spool.tile([128, 10], F32, name="outh", tag="outh")
                tmp = spool.tile([128, 10], F32, name="tmph", tag="tmph")
                nc.vector.tensor_scalar_mul(
                    tmp[:], t["fcb"][:, 0:10], pooled[:, 0:1]
                )
                nc.vector.tensor_add(outh[:], tmp[:], t["fcb"][:, 10:20])
                nc.sync.dma_start(out_d[s][:, h * 10 : (h + 1) * 10], outh[:])

            def emit_fc(s, t):
                pooled = spool.tile([128, 2], F32, name="pooled", tag="pooled")
                nc.vector.tensor_reduce(
                    pooled[:],
                    t["acc"][:].rearrange("p (h o) -> p h o", h=2),
                    axis=mybir.AxisListType.X,
                    op=ADD,
                )
                outsb = spool.tile([128, 20], F32, name="outsb", tag="outsb")
                for h in range(2):
                    tmp = spool.tile([128, 10], F32, name="tmp", tag="tmp")
                    nc.vector.tensor_scalar_mul(
                        tmp[:], t["fcb"][:, 0:10], pooled[:, h : h + 1]
                    )
                    nc.vector.tensor_add(
                        outsb[:, h * 10 : h * 10 + 10], tmp[:], t["fcb"][:, 10:20]
                    )
                nc.sync.dma_start(out_d[s], outsb[:])

            # software-pipelined emission. Per sample, phase P1 interleaves
            # conv2[k] / conv1(s+1)[k-2] / conv3h0[k-2] so the PE always has
            # matmul work queued while the ACT/DVE drains catch up; phase P2
            # runs conv3h1 alone (drains alternate ACT/DVE).
            T[0] = emit_loads_xim(0)
            emit_loads_rest(0, T[0])
            border_memset(pads1[1], nc.gpsimd)
            border_memset(pads2[1], nc.gpsimd)
            border_memset(pads2[1], nc.gpsimd, OB2)
            emit_conv1_pair(T[0], 0)
            emit_conv1_pair(T[0], 2)
            border_memset(pads2[0], nc.vector)
            border_memset(pads2[0], nc.vector, OB2)
            for s in range(SPC):
                t = T[s]
                t["acc"] = spool.tile([128, 16], F32, name="acc", tag="acc")
                if s + 1 < SPC:
                    if s == 0:
                        T[1] = emit_loads_xim(1)
                    emit_loads_rest(s + 1, T[s + 1])
                for k in range(10):
                    if k <= 7:
                        emit_conv2_chunk(t, *RCHUNKS[k], k)
                    if s == 0 and k <= 1:
                        emit_conv1_pair(t, 4 + 2 * k)
                    if k == 0 and s >= 1:
                        emit_fc(s - 1, T[s - 1])
                        T[s - 1] = None
                    if 2 <= k:
                        if s + 1 < SPC and k % 2 == 0:
                            emit_conv1_pair(T[s + 1], k - 2)
                        emit_conv3_chunk(t, 0, k - 2, "dve")
                if s == SPC - 1:
                    emit_fc_half(s, t, 0)
                if s + 2 < SPC:
                    # prefetch the NEXT-next sample's xim a full phase early
                    # so conv1 pairs never wait on DMA at sample boundaries
                    T[s + 2] = emit_loads_xim(s + 2)
                for k in range(8):
                    # ACT only takes two mid drains; keeps it free when the
                    # next sample's P1 conv2 drains start
                    emit_conv3_chunk(t, 1, k, "act" if k in (1, 3) else "dve")
            emit_fc_half(SPC - 1, T[SPC - 1], 1)
    nc.compile()
    return nc


def prep_inputs(x, conv1_weight, conv2_weight, conv3_weight, fc_weight,
                bias1, bias2, bias3, bias4):
    """Host-side layout prep (pure data movement + static scaling)."""
    import ml_dtypes

    f = np.float32
    bf = ml_dtypes.bfloat16
    f8 = ml_dtypes.float8_e4m3fn

    def q8(a):
        return np.clip(a, -240.0, 240.0).astype(f8)

    x = np.asarray(x, f)
    padx = np.zeros((B, 3, 66, 66), f)
    padx[:, :, 1:65, 1:65] = x
    padflat = padx.reshape(B, 3, NPIX1)
    D66 = [(dy - 1) * 66 + (dx - 1) for dy in range(3) for dx in range(3)]
    xim = np.zeros((B, 27, NPIX1), f)
    for s, d in enumerate(D66):
        lo = max(0, -d)
        hi = min(NPIX1, NPIX1 - d)
        xim[:, s * 3 : s * 3 + 3, lo:hi] = padflat[:, :, lo + d : hi + d]

    w1 = q8(
        np.asarray(conv1_weight, f).transpose(0, 3, 4, 2, 1).reshape(B, 27, 64)
        * np.float32(A1)
    )
    s2 = np.float32(A2 / A1)
    s3 = np.float32(A3 / A2)
    # conv2 DR weights: [b, k:128, j:3, o:2, m:128]
    w2n = np.asarray(conv2_weight, f).transpose(0, 2, 3, 4, 1)  # [b,ci,ky,kx,co]
    w2p = np.zeros((B, 128, 3, 2, 128), f)
    for j in range(3):
        w2p[:, 0:64, j, 0] = s2 * w2n[:, :, 0, j]   # dy=0 (top half)
        w2p[:, 64:128, j, 0] = s2 * w2n[:, :, 1, j]  # dy=1 (shifted bottom)
        w2p[:, 0:64, j, 1] = s2 * w2n[:, :, 2, j]   # dy=2 (pair axis, top)
    w2 = q8(w2p.reshape(B, 128, 3 * 2 * 128))
    # conv3 weights: per half h: 3 DR pairs [j][o:2][m:128] then 3 singles
    w3n = np.asarray(conv3_weight, f).transpose(0, 2, 3, 4, 1)  # [b,ci,ky,kx,co]
    w3p = np.zeros((B, 128, 2, 1152), f)
    for hh in range(2):
        co = slice(hh * 128, (hh + 1) * 128)
        for j in range(3):
            w3p[:, :, hh, j * 256 : j * 256 + 128] = s3 * w3n[:, :, 0, j, co]
            w3p[:, :, hh, j * 256 + 128 : j * 256 + 256] = s3 * w3n[:, :, 2, j, co]
        # 4th DR pair: (1,0) with (1,1) via the second pad2 copy
        w3p[:, :, hh, 768:896] = s3 * w3n[:, :, 1, 0, co]
        w3p[:, :, hh, 896:1024] = s3 * w3n[:, :, 1, 1, co]
        # plain single: (1,2)
        w3p[:, :, hh, 1024:1152] = s3 * w3n[:, :, 1, 2, co]
    w3 = q8(w3p.reshape(B, 128, 2 * 1152))
    sm = np.zeros((B, 128, 24), f)
    sm[:, 0:64, 0] = np.asarray(bias1, f) * np.float32(A1)
    sm[:, :, 1] = np.asarray(bias2, f) * np.float32(A2)
    sm[:, :, 2:4] = (
        np.asarray(bias3, f).reshape(B, 2, 128).transpose(0, 2, 1)
        * np.float32(A3)
    )
    fcs = np.asarray(fc_weight, f)[:, 0, :] / np.float32(H * W * A3)
    sm[:, :, 4:14] = np.repeat(fcs[:, None, :], 128, axis=1)
    sm[:, :, 14:24] = np.repeat(np.asarray(bias4, f)[:, None, :], 128, axis=1)
    return (q8(xim), w1, w2, w3, np.ascontiguousarray(sm))


def _ensure_ntff_hook():
    """The agent image's `antenv` lacks `axon_hooks`, so boot skipped
    registering the NTFF profiling hook and bass_utils would crash
    importing it under BASS_TRACE. Install the same ctypes-based hook
    trn_boot would have registered; degrade to hook=None on any failure
    (bass_utils then skips tracing and still runs)."""
    import types

    if "antenv.axon_hooks" in sys.modules:
        return
    try:
        import antenv

        mod = types.ModuleType("antenv.axon_hooks")
        _state = {"hook": None}
        mod.set_axon_ntff_profile_hook = lambda h: _state.__setitem__("hook", h)
        mod.get_axon_ntff_profile_hook = lambda: _state["hook"]
        sys.modules["antenv.axon_hooks"] = mod
        antenv.axon_hooks = mod
        try:
            from trn_agent_boot.trn_boot import _ntff_profile_via_ctypes

            hook = _ntff_profile_via_ctypes("/opt/axon/libaxon_pjrt.so")
            if hook is not None:
                _state["hook"] = hook
        except Exception:
            pass
    except Exception:
        pass


_NC_CACHE = {}
LAST_RESULTS = None


def kernel(x, conv1_weight, conv2_weight, conv3_weight, fc_weight,
           bias1, bias2, bias3, bias4):
    global LAST_RESULTS
    xim, w1, w2, w3, sm = prep_inputs(
        x, conv1_weight, conv2_weight, conv3_weight, fc_weight,
        bias1, bias2, bias3, bias4,
    )
    if "nc" not in _NC_CACHE:
        _NC_CACHE["nc"] = build_nc()
    nc = _NC_CACHE["nc"]

    in_maps = []
    for c in range(N_CORES):
        sl = slice(c * SPC, (c + 1) * SPC)
        in_maps.append(
            {
                "xim": np.ascontiguousarray(xim[sl]),
                "w1": np.ascontiguousarray(w1[sl]),
                "w2": np.ascontiguousarray(w2[sl]),
                "w3": np.ascontiguousarray(w3[sl]),
                "sm": np.ascontiguousarray(sm[sl]),
            }
        )
    _ensure_ntff_hook()
    try:
        res = run_bass_kernel_spmd(nc, in_maps, list(range(N_CORES)))
    except Exception:
        # a previous crash can leave the NeuronCore exec unit wedged;
        # reset through the axon plugin and retry once
        try:
            import ctypes

            lib = ctypes.CDLL("/opt/axon/libaxon_pjrt.so")
            lib.axon_reset.restype = ctypes.c_int64
            lib.axon_reset()
        except Exception:
            pass
        res = run_bass_kernel_spmd(nc, in_maps, list(range(N_CORES)))
    LAST_RESULTS = res
    outs = []
    for c in range(N_CORES):
        o = np.asarray(res.results[c]["out"], np.float32)  # [SPC, 128, 20]
        outs.append(o.reshape(SPC, 128, 2, 10).transpose(0, 2, 1, 3).reshape(SPC, 256, 10))
    return np.concatenate(outs, axis=0)
